# revision 23
# baseline (speedup 1.0000x reference)
"""CGCNN (no BN) message-passing GNN on 8 Trainium2 NeuronCores.

Strategy (self-contained; shapes hardcoded from the problem spec):
 - Nodes are permuted on the host into 392 blocks of 128 slots, balancing
   per-block in-edge counts. Cores own 49 contiguous blocks (6272 slots).
 - Edges are owned by the core that owns their destination block; within a
   block, edges are split by source-slot half (<32768 vs >=32768) so each
   128-edge tile gathers from a single int16-indexable table view, then
   padded to a uniform (TL, TH) tile count per block so all 8 cores run one
   SPMD program.
 - Inputs are minimized so the steady-state (device-resident) run needs no
   re-upload: edge gaussian features are computed ON DEVICE from a per-edge
   distance shipped as a bf16 hi/lo split ([2, S]); the initial node
   features x0 = (embedding @ emb_w)[z] are built ON DEVICE from per-slot z
   values via a one-hot matmul against the 100x128 EW table; the per-graph
   mean pool runs ON DEVICE via one-hot matmuls into two PSUM accumulators,
   so the only output is [256, 128] partial graph sums per core.
 - Per 128-edge tile on device: dma_gather (SBUF source, transposed) pulls
   x[src] / x[dst] columns in channel-major bf16; a 1-wide PE matmul
   broadcasts d to 101 partitions; ACT Square+Exp produce the gaussian
   features (with a constant-1 bias row); three PE matmuls accumulate the
   conv pre-activation in PSUM; ACT computes sigmoid/softplus via Exp/Ln
   only (one act table); DVE builds a one-hot dst matrix which PE uses to
   segment-sum messages into the block accumulator.
 - LayerNorm + residual + softplus per block in f32; updated x is written to
   a bf16 local table and AllGathered into the full bf16 gather table
   between layers (and once for x0 before layer 0).
 - The tiny pooled-MLP head runs on the host in f32 (0.01% of FLOPs).
 - Execution goes through a jit'd shard_map over the 8 axon devices with
   inputs device_put ONCE; warm executions (inputs resident, outputs
   donated-zeros) are timed to report steady-state HW execution wall time.
"""

import os as _os
import time as _time
import numpy as np
import ml_dtypes

import jax
from jax.sharding import Mesh, PartitionSpec, NamedSharding

try:
    from jax.experimental.shard_map import shard_map
except Exception:
    from jax import shard_map as _shard_map_mod  # jax >= 0.8 namespace
    shard_map = (_shard_map_mod.shard_map
                 if hasattr(_shard_map_mod, "shard_map") else _shard_map_mod)

import concourse.bass as bass
import concourse.tile as tile
from concourse import bacc, mybir
from concourse.bass_utils import run_bass_kernel_spmd

BF16 = ml_dtypes.bfloat16

# Problem constants
N_NODES, N_EDGES, NODE_D, EDGE_D, EMB_D, N_GRAPHS = 50000, 800000, 128, 100, 92, 256
N_CONV, FC_D, N_FC, CUTOFF = 3, 128, 2, 6.0

LAST_RESULTS = None        # BassKernelResults of the most recent fallback run
LAST_WARM_NS = None        # amortized wall-ns per execution (pipelined batch)
LAST_WARM_ALL = None       # single-call warm latencies (ns, incl relay RTT)
LAST_BATCH_NS = None       # {K: amortized ns/iter} for pipelined batches
LAST_MARGINAL_NS = None    # marginal ns/iter between the two batch sizes

N_CORES = 8
SLOTS = 50176              # 392 blocks * 128
BLOCKS = SLOTS // 128      # 392
NBLK = BLOCKS // N_CORES   # 49 blocks per core
CORE_SLOTS = NBLK * 128    # 6272
LO_SLOTS = 32768           # slots gatherable from the low table view
CHUNK = 2                  # blocks processed per gather chunk
WARM_ITERS = int(_os.environ.get("KERNEL_WARM_ITERS", "3"))


# --------------------------------------------------------------------------
# Device program
# --------------------------------------------------------------------------

def build_nc(TL, TH, nblk=NBLK, ranks=BLOCKS, n_cores=N_CORES,
             lo_ranks=LO_SLOTS // 128, emit_xfin=False):
    """Build the SPMD Bass program. TL/TH = low/high tiles per block."""
    TPB = TL + TH                 # tiles per block
    NT = nblk * TPB               # tiles per core
    S = NT * 128                  # edge slots per core
    SLO = nblk * TL * 128
    SHI = nblk * TH * 128
    core_slots = nblk * 128
    f32, bf, i16 = mybir.dt.float32, mybir.dt.bfloat16, mybir.dt.int16
    AF = mybir.ActivationFunctionType

    offs = np.linspace(0.0, CUTOFF, EDGE_D, dtype=np.float64)
    coeff = float(-0.5 / (offs[1] - offs[0]) ** 2)

    nc = bacc.Bacc("TRN2", target_bir_lowering=False, debug=False,
                   num_devices=n_cores)

    # inputs
    eps = 1e-5
    ixi_d = nc.dram_tensor("ixi", [128, S // 16], i16, kind="ExternalInput").ap()
    ixlo_d = nc.dram_tensor("ixlo", [128, SLO // 16], i16, kind="ExternalInput").ap()
    ixhi_d = nc.dram_tensor("ixhi", [128, SHI // 16], i16, kind="ExternalInput").ap()
    dst_d = nc.dram_tensor("dstv", [128, NT], f32, kind="ExternalInput").ap()
    dvec_d = nc.dram_tensor("dvec", [2, S], bf, kind="ExternalInput").ap()
    zval_d = nc.dram_tensor("zval", [1, core_slots], f32, kind="ExternalInput").ap()
    glo_d = nc.dram_tensor("gidlo", [128, nblk], f32, kind="ExternalInput").ap()
    ghi_d = nc.dram_tensor("gidhi", [128, nblk], f32, kind="ExternalInput").ap()
    ew_d = nc.dram_tensor("ew", [128, 128], f32, kind="ExternalInput").ap()
    pidx_d = nc.dram_tensor("pidx", [128, 1], f32, kind="ExternalInput").ap()
    iota_d = nc.dram_tensor("iota", [128, 128], bf, kind="ExternalInput").ap()
    wxi_d = nc.dram_tensor("wxi", [128, N_CONV, 256], bf, kind="ExternalInput").ap()
    wxj_d = nc.dram_tensor("wxj", [128, N_CONV, 256], bf, kind="ExternalInput").ap()
    wea_d = nc.dram_tensor("wea", [101, N_CONV, 256], bf, kind="ExternalInput").ap()
    g_d = nc.dram_tensor("lng", [128, N_CONV, 128], f32, kind="ExternalInput").ap()
    b_d = nc.dram_tensor("lnb", [128, N_CONV, 128], f32, kind="ExternalInput").ap()
    dsel_d = nc.dram_tensor("dsel", [2, 101], bf, kind="ExternalInput").ap()
    noff_d = nc.dram_tensor("negoff", [101, 1], f32, kind="ExternalInput").ap()

    # internal DRAM: x master copies (f32) for layers 0..2 inputs
    xmast = [
        nc.dram_tensor(f"xmast{i}", [core_slots, 128], f32, kind="Internal").ap()
        for i in range(N_CONV)
    ]
    xout = [
        nc.dram_tensor(f"xout{i}", [core_slots, 128], bf, kind="Internal").ap()
        for i in range(N_CONV)
    ]
    xall = [
        nc.dram_tensor(f"xall{i}", [n_cores * core_slots, 128], bf,
                       kind="Internal", addr_space="Shared").ap()
        for i in range(N_CONV)
    ]
    # output: per-core partial graph sums (lo graphs 0-127, hi graphs 128-255)
    gsum_d = nc.dram_tensor("gsum", [256, 128], f32, kind="ExternalOutput").ap()
    if emit_xfin:
        xfin_d = nc.dram_tensor("xfin", [core_slots, 128], f32,
                                kind="ExternalOutput").ap()

    rg = [list(range(n_cores))]

    with tile.TileContext(nc) as tc:
        with (
            tc.tile_pool(name="persist", bufs=1) as persist,
            tc.tile_pool(name="prol", bufs=2) as prol_p,
            tc.tile_pool(name="gxi", bufs=2) as gxi_p,
            tc.tile_pool(name="glo", bufs=2) as glo_p,
            tc.tile_pool(name="ghi", bufs=2) as ghi_p,
            tc.tile_pool(name="dstr", bufs=2) as dstr_p,
            tc.tile_pool(name="eat", bufs=3) as ea_p,
            tc.tile_pool(name="idx", bufs=2) as idx_p,
            tc.tile_pool(name="small", bufs=4) as small_p,
            tc.tile_pool(name="xio", bufs=2) as xio_p,
            tc.tile_pool(name="stats", bufs=2) as stats_p,
            tc.tile_pool(name="zc", bufs=2, space="PSUM") as zc_p,
            tc.tile_pool(name="agg", bufs=2, space="PSUM") as agg_p,
            tc.tile_pool(name="dps", bufs=2, space="PSUM") as dps_p,
            tc.tile_pool(name="gacc", bufs=1, space="PSUM") as gacc_p,
        ):
            # persistent SBUF
            tab_s = persist.tile([128, ranks * 128], bf)
            # single local x table: per-chunk xi gathers only read the
            # chunk's own (not-yet-rewritten) blocks, so layer l+1's values
            # can overwrite block slices in place (range-level hazards keep
            # this correct without a second table).
            loc_s = persist.tile([128, nblk, 128], bf, tag="loca")
            dst_s = persist.tile([128, NT], f32)
            iota_s = persist.tile([128, 128], bf)
            wxi_s = persist.tile([128, N_CONV, 256], bf)
            wxj_s = persist.tile([128, N_CONV, 256], bf)
            wea_s = persist.tile([101, N_CONV, 256], bf)
            g_s = persist.tile([128, N_CONV, 128], f32)
            b_s = persist.tile([128, N_CONV, 128], f32)
            ew_s = persist.tile([128, 128], f32, tag="ew")
            pidx_s = persist.tile([128, 1], f32, tag="pidx")
            glo_s = persist.tile([128, nblk], f32, tag="gidlo")
            ghi_s = persist.tile([128, nblk], f32, tag="gidhi")
            dsel_s = persist.tile([2, 101], bf, tag="dsel")
            noff_s = persist.tile([101, 1], f32, tag="noff")
            eps_s = persist.tile([128, 1], f32)
            ones_s = persist.tile([128, 1], f32)

            nc.sync.dma_start(dst_s[:], dst_d)
            nc.sync.dma_start(iota_s[:], iota_d)
            nc.sync.dma_start(wxi_s[:], wxi_d)
            nc.sync.dma_start(wxj_s[:], wxj_d)
            nc.sync.dma_start(wea_s[:], wea_d)
            nc.sync.dma_start(g_s[:], g_d)
            nc.sync.dma_start(b_s[:], b_d)
            nc.sync.dma_start(ew_s[:], ew_d)
            nc.sync.dma_start(pidx_s[:], pidx_d)
            nc.sync.dma_start(glo_s[:], glo_d)
            nc.sync.dma_start(ghi_s[:], ghi_d)
            nc.sync.dma_start(dsel_s[:], dsel_d)
            nc.sync.dma_start(noff_s[:], noff_d)
            nc.vector.memset(eps_s[:], eps)
            nc.vector.memset(ones_s[:], 1.0)

            def exchange(i, loc_tile):
                """loc_tile (bf16 local x table) -> AllGather -> tab_s."""
                nc.sync.dma_start(
                    xout[i].rearrange("(r p) c -> p r c", p=128), loc_tile[:])
                nc.gpsimd.collective_compute(
                    "AllGather", mybir.AluOpType.bypass,
                    replica_groups=rg,
                    ins=[xout[i][:]], outs=[xall[i][:]])
                nc.sync.dma_start(
                    tab_s[:].rearrange("p (r c) -> p r c", c=128),
                    xall[i].rearrange("(r p) c -> p r c", p=128))

            # ---- prologue: x0 = EW[z] per block, via one-hot matmul ------
            for blk in range(nblk):
                zrow = prol_p.tile([128, 128], f32, tag="zrow")
                nc.sync.dma_start(
                    zrow[:],
                    zval_d[0:1, blk * 128:(blk + 1) * 128].to_broadcast((128, 128)))
                ohT = prol_p.tile([128, 128], f32, tag="ohT")
                nc.vector.tensor_tensor(
                    out=ohT[:], in0=pidx_s[:].to_broadcast((128, 128)),
                    in1=zrow[:], op=mybir.AluOpType.is_equal)
                x0_ps = agg_p.tile([128, 128], f32, tag="agg")
                nc.tensor.matmul(x0_ps[:], ohT[:], ew_s[:], start=True, stop=True)
                x0_sb = xio_p.tile([128, 128], f32, tag="xnew")
                nc.vector.tensor_copy(out=x0_sb[:], in_=x0_ps[:])
                nc.scalar.activation(loc_s[:, blk, :], x0_ps[:], AF.Copy)
                nc.sync.dma_start(xmast[0][blk * 128:(blk + 1) * 128, :], x0_sb[:])
            exchange(0, loc_s)

            n_chunks = (nblk + CHUNK - 1) // CHUNK
            tab_lo_view = tab_s[:, : lo_ranks * 128]
            tab_hi_view = tab_s[:, lo_ranks * 128:]
            for layer in range(N_CONV):
                last = layer == N_CONV - 1
                loc_flat = loc_s.rearrange("p r c -> p (r c)")
                if last:
                    gacc_lo = gacc_p.tile([128, 128], f32, tag="gacclo")
                    gacc_hi = gacc_p.tile([128, 128], f32, tag="gacchi")

                for ch in range(n_chunks):
                    b0 = ch * CHUNK
                    nb = min(CHUNK, nblk - b0)  # blocks in this chunk
                    n_ti = nb * TPB             # xi tiles in chunk
                    n_tl = nb * TL
                    n_th = nb * TH

                    # ---- per-chunk loads -------------------------------
                    ixi_t = idx_p.tile([128, CHUNK * TPB * 8], i16, tag="ixi")
                    ixlo_t = idx_p.tile([128, CHUNK * TL * 8], i16, tag="ixlo")
                    ixhi_t = idx_p.tile([128, CHUNK * TH * 8], i16, tag="ixhi")
                    d_t = dstr_p.tile([2, CHUNK * TPB * 128], bf, tag="dv")
                    c0 = b0 * TPB * 8
                    nc.sync.dma_start(ixi_t[:, :n_ti * 8],
                                      ixi_d[:, c0:c0 + n_ti * 8])
                    nc.sync.dma_start(ixlo_t[:, :n_tl * 8],
                                      ixlo_d[:, b0 * TL * 8: b0 * TL * 8 + n_tl * 8])
                    nc.sync.dma_start(ixhi_t[:, :n_th * 8],
                                      ixhi_d[:, b0 * TH * 8: b0 * TH * 8 + n_th * 8])
                    nc.sync.dma_start(d_t[:, :n_ti * 128],
                                      dvec_d[:, b0 * TPB * 128:
                                             (b0 * TPB + n_ti) * 128])

                    # ---- gathers (SBUF-source, transposed, bf16) -------
                    xi_g = gxi_p.tile([128, 1, CHUNK * TPB * 128], bf, tag="xi")
                    lo_g = glo_p.tile([128, 1, CHUNK * TL * 128], bf, tag="lo")
                    hi_g = ghi_p.tile([128, 1, CHUNK * TH * 128], bf, tag="hi")
                    loc_view = loc_flat[:, b0 * 128:(b0 + nb) * 128]
                    nc.gpsimd.dma_gather(
                        xi_g[:, :, :n_ti * 128], loc_view, ixi_t[:, :n_ti * 8],
                        n_ti * 128, n_ti * 128, 128,
                        transpose=True, sbuf_tokens_per_rank=128,
                        sbuf_free_dim_per_rank=256, single_packet=False)
                    nc.gpsimd.dma_gather(
                        lo_g[:, :, :n_tl * 128], tab_lo_view, ixlo_t[:, :n_tl * 8],
                        n_tl * 128, n_tl * 128, 128,
                        transpose=True, sbuf_tokens_per_rank=128,
                        sbuf_free_dim_per_rank=256, single_packet=False)
                    nc.gpsimd.dma_gather(
                        hi_g[:, :, :n_th * 128], tab_hi_view, ixhi_t[:, :n_th * 8],
                        n_th * 128, n_th * 128, 128,
                        transpose=True, sbuf_tokens_per_rank=128,
                        sbuf_free_dim_per_rank=256, single_packet=False)

                    # ---- per-block compute -----------------------------
                    for bi in range(nb):
                        blk = b0 + bi
                        agg = agg_p.tile([128, 128], f32, tag="agg")
                        for t in range(TPB):
                            is_lo = t < TL
                            xi_sl = xi_g[:, 0, (bi * TPB + t) * 128:
                                         (bi * TPB + t + 1) * 128]
                            if is_lo:
                                xj_sl = lo_g[:, 0, (bi * TL + t) * 128:
                                             (bi * TL + t + 1) * 128]
                            else:
                                th = t - TL
                                xj_sl = hi_g[:, 0, (bi * TH + th) * 128:
                                             (bi * TH + th + 1) * 128]

                            # edge features: ea = exp(coeff*(d-off)^2), row
                            # 100 == 1 (conv bias row). d arrives as a bf16
                            # hi/lo split; dsel sums the halves into f32 and
                            # zeroes row 100 so Square+Exp leave it at 1.
                            dps = dps_p.tile([101, 128], f32, tag="dps")
                            nc.tensor.matmul(
                                dps[:], dsel_s[:],
                                d_t[:, (bi * TPB + t) * 128:
                                    (bi * TPB + t + 1) * 128],
                                start=True, stop=True)
                            nc.scalar.activation(dps[:], dps[:], AF.Square,
                                                 bias=noff_s[:])
                            ea_sl = ea_p.tile([101, 128], bf, tag="ea")
                            nc.scalar.activation(ea_sl[:], dps[:], AF.Exp,
                                                 scale=coeff)

                            zc = zc_p.tile([128, 256], f32, tag="zc")
                            nc.tensor.matmul(zc[:], xi_sl, wxi_s[:, layer, :],
                                             start=True, stop=False)
                            nc.tensor.matmul(zc[:], xj_sl, wxj_s[:, layer, :],
                                             start=False, stop=False)
                            nc.tensor.matmul(zc[:], ea_sl[:], wea_s[:, layer, :],
                                             start=False, stop=True)

                            sel = small_p.tile([128, 128], bf, tag="sel")
                            nc.vector.tensor_scalar(
                                out=sel[:], in0=iota_s[:],
                                scalar1=dst_s[:, blk * TPB + t: blk * TPB + t + 1],
                                scalar2=None, op0=mybir.AluOpType.is_equal)

                            # zc holds [-z1 | z2] (z1-half weights sign-flipped
                            # on host).  msg = softplus(z2) * sigmoid(z1)
                            #          = ln(1+e^{z2}) / (1 + e^{-z1})
                            ez = small_p.tile([128, 256], f32, tag="ez")
                            nc.scalar.activation(ez[:], zc[:], AF.Exp)
                            sp = small_p.tile([128, 128], bf, tag="sp")
                            nc.scalar.activation(sp[:], ez[:, 128:256], AF.Ln,
                                                 bias=ones_s[:])
                            u1 = small_p.tile([128, 128], f32, tag="u1")
                            nc.vector.tensor_scalar(
                                out=u1[:], in0=ez[:, 0:128], scalar1=1.0,
                                scalar2=None, op0=mybir.AluOpType.add)
                            rcp = small_p.tile([128, 128], f32, tag="rcp")
                            nc.vector.reciprocal(rcp[:], u1[:])
                            msg = small_p.tile([128, 128], bf, tag="msg")
                            nc.vector.tensor_mul(msg[:], sp[:], rcp[:])

                            nc.tensor.matmul(agg[:], sel[:], msg[:],
                                             start=(t == 0), stop=(t == TPB - 1))

                        # ---- block epilogue: LN + residual + softplus --
                        xold = xio_p.tile([128, 128], f32, tag="xold")
                        nc.sync.dma_start(
                            xold[:], xmast[layer][blk * 128:(blk + 1) * 128, :])

                        st = stats_p.tile([128, 6], f32, tag="bn")
                        nc.vector.bn_stats(out=st[:], in_=agg[:])
                        mv = stats_p.tile([128, 2], f32, tag="mv")
                        nc.vector.bn_aggr(out=mv[:], in_=st[:])
                        # rstd = exp(-0.5 * ln(var + eps))
                        lnv = stats_p.tile([128, 1], f32, tag="lnv")
                        nc.scalar.activation(lnv[:], mv[:, 1:2], AF.Ln,
                                             bias=eps_s[:])
                        rstd = stats_p.tile([128, 1], f32, tag="rstd")
                        nc.scalar.activation(rstd[:], lnv[:], AF.Exp,
                                             scale=-0.5)

                        xn = xio_p.tile([128, 128], f32, tag="xn")
                        nc.vector.tensor_scalar(
                            out=xn[:], in0=agg[:], scalar1=mv[:, 0:1],
                            scalar2=rstd[:], op0=mybir.AluOpType.subtract,
                            op1=mybir.AluOpType.mult)
                        nc.vector.tensor_mul(xn[:], xn[:], g_s[:, layer, :])
                        nc.vector.tensor_add(xn[:], xn[:], b_s[:, layer, :])
                        nc.vector.tensor_add(xn[:], xn[:], xold[:])

                        # softplus(xn) = ln(1 + e^{xn})
                        exn = xio_p.tile([128, 128], f32, tag="exn")
                        nc.scalar.activation(exn[:], xn[:], AF.Exp)
                        xnew = xio_p.tile([128, 128], f32, tag="xnew")
                        nc.scalar.activation(xnew[:], exn[:], AF.Ln,
                                             bias=ones_s[:])
                        if not last:
                            # bf16 copy into next layer's local gather table
                            nc.scalar.activation(loc_s[:, blk, :], xnew[:],
                                                 AF.Copy)
                            nc.sync.dma_start(
                                xmast[layer + 1][blk * 128:(blk + 1) * 128, :],
                                xnew[:])
                        else:
                            # on-device graph pooling: one-hot over graph ids
                            gsl = stats_p.tile([128, 128], f32, tag="gsl")
                            nc.vector.tensor_scalar(
                                out=gsl[:], in0=iota_s[:],
                                scalar1=glo_s[:, blk:blk + 1],
                                scalar2=None, op0=mybir.AluOpType.is_equal)
                            gsh = stats_p.tile([128, 128], f32, tag="gsh")
                            nc.vector.tensor_scalar(
                                out=gsh[:], in0=iota_s[:],
                                scalar1=ghi_s[:, blk:blk + 1],
                                scalar2=None, op0=mybir.AluOpType.is_equal)
                            nc.tensor.matmul(gacc_lo[:], gsl[:], xnew[:],
                                             start=(blk == 0),
                                             stop=(blk == nblk - 1))
                            nc.tensor.matmul(gacc_hi[:], gsh[:], xnew[:],
                                             start=(blk == 0),
                                             stop=(blk == nblk - 1))
                            if emit_xfin:
                                nc.sync.dma_start(
                                    xfin_d[blk * 128:(blk + 1) * 128, :],
                                    xnew[:])

                # ---- exchange (layers 0,1): slice -> AllGather -> table
                if not last:
                    exchange(layer + 1, loc_s)
                else:
                    gs_lo = xio_p.tile([128, 128], f32, tag="gslo")
                    nc.vector.tensor_copy(out=gs_lo[:], in_=gacc_lo[:])
                    nc.sync.dma_start(gsum_d[0:128, :], gs_lo[:])
                    gs_hi = xio_p.tile([128, 128], f32, tag="gshi")
                    nc.vector.tensor_copy(out=gs_hi[:], in_=gacc_hi[:])
                    nc.sync.dma_start(gsum_d[128:256, :], gs_hi[:])

    nc.compile()
    return nc


# --------------------------------------------------------------------------
# Host preprocessing
# --------------------------------------------------------------------------

def _softplus(x):
    return np.log1p(np.exp(-np.abs(x))) + np.maximum(x, 0.0)


def preprocess(z, R, edge_index, batch, embedding, emb_w, emb_b, conv_w,
               conv_b, ln_g, ln_b, n_nodes=N_NODES, n_cores=N_CORES,
               nblk=NBLK, lo_slots=LO_SLOTS, edge_d=EDGE_D):
    blocks = n_cores * nblk
    slots = blocks * 128
    core_slots = nblk * 128
    lo_blocks = lo_slots // 128
    n_edges = edge_index.shape[1]
    src = np.asarray(edge_index[0], np.int64)
    dst = np.asarray(edge_index[1], np.int64)

    # edge distances on host (cheap); smearing happens on device
    Rf = np.asarray(R, np.float32)
    d = np.linalg.norm(Rf[src] - Rf[dst], axis=-1).astype(np.float32)  # [E]

    # node permutation: balance per-block in-degrees; L = orig nodes < lo_slots
    islo_e = src < lo_slots
    a = np.bincount(dst[islo_e], minlength=n_nodes)
    b = np.bincount(dst[~islo_e], minlength=n_nodes)
    w = a + b
    # L-nodes -> slots [0, lo_slots); rest -> [lo_slots, slots)
    ordL = np.argsort(-w[:lo_slots], kind="stable")
    ordH = np.argsort(-w[lo_slots:], kind="stable") + lo_slots
    perm = np.full(n_nodes, -1, np.int64)
    perm[ordL] = _snake_slots(ordL.size, lo_blocks)
    perm[ordH] = _snake_slots(ordH.size, blocks - lo_blocks) + lo_slots
    assert perm.min() >= 0

    es, ed = perm[src], perm[dst]
    blk = ed // 128

    lo_cnt = np.bincount(blk[islo_e], minlength=blocks)
    hi_cnt = np.bincount(blk[~islo_e], minlength=blocks)
    TL = int(-(-lo_cnt.max() // 128))
    TH = int(-(-hi_cnt.max() // 128))
    TPB = TL + TH
    S = nblk * TPB * 128

    # edge slot assignment: within block, lows first then highs
    key = blk * 2 + (~islo_e).astype(np.int64)
    eorder = np.argsort(key, kind="stable")
    ks = key[eorder]
    # position within each (block, half) run
    runstart = np.r_[0, np.flatnonzero(np.diff(ks)) + 1]
    runid = np.zeros(n_edges, np.int64)
    runid[runstart[1:]] = 1
    runid = np.cumsum(runid)
    pos = np.arange(n_edges) - runstart[runid]
    eb = ks // 2
    ehalf = ks % 2
    base = eb * TPB * 128 + ehalf * (TL * 128)
    eslot_g = base + pos                       # global edge slot (per full graph)
    # per-core arrays
    core_of = eb // nblk
    eslot = eslot_g - core_of * (nblk * TPB * 128)

    ixi = np.zeros((n_cores, S), np.int16)
    ixlo = np.zeros((n_cores, nblk * TL * 128), np.int16)
    ixhi = np.zeros((n_cores, nblk * TH * 128), np.int16)
    dstv = np.full((n_cores, nblk * TPB, 128), -1.0, np.float32)
    dvec = np.zeros((n_cores, 2, S), BF16)

    e_src = es[eorder]
    e_dst = ed[eorder]
    e_lo = ehalf == 0
    d_o = d[eorder]
    dc_o = d_o.astype(BF16)
    df_o = (d_o - dc_o.astype(np.float32)).astype(BF16)

    for c in range(n_cores):
        m = core_of == c
        sl = eslot[m]
        # xi: dst local to the chunk's 2-block view
        dloc = (e_dst[m] - c * core_slots) % (CHUNK * 128)
        ixi[c][sl] = dloc.astype(np.int16)
        # xj
        mlo = m & e_lo
        mhi = m & ~e_lo
        slo_ = eslot[mlo]
        # map edge slot -> position in the lo stream
        bb = slo_ // (TPB * 128)
        off = slo_ - bb * (TPB * 128)
        ixlo[c][bb * TL * 128 + off] = e_src[mlo].astype(np.int16)
        shi_ = eslot[mhi]
        bb = shi_ // (TPB * 128)
        off = shi_ - bb * (TPB * 128) - TL * 128
        ixhi[c][bb * TH * 128 + off] = (e_src[mhi] - lo_slots).astype(np.int16)
        # dst one-hot value, distance hi/lo split
        dstv[c].reshape(-1)[sl] = (e_dst[m] % 128).astype(np.float32)
        dvec[c][0][sl] = dc_o[m]
        dvec[c][1][sl] = df_o[m]

    def wrap16(arr):
        # [S] int16 -> [128, S/16], idx i at (i%16, i//16), tiled to 128 parts
        t = arr.reshape(-1, 16).T
        return np.tile(t, (8, 1)).copy()

    # per-slot host arrays
    inv = np.full(slots, -1, np.int64)
    inv[perm] = np.arange(n_nodes)
    valid = inv >= 0
    zsl = np.full(slots, -1.0, np.float32)
    zsl[valid] = np.asarray(z, np.int64)[inv[valid]].astype(np.float32)
    gid = np.full(slots, -1, np.int64)
    gid[valid] = np.asarray(batch, np.int64)[inv[valid]]
    gidlo = np.where((gid >= 0) & (gid < 128), gid, -1).astype(np.float32)
    gidhi = np.where(gid >= 128, gid - 128, -1).astype(np.float32)

    # EW table (f32, padded to 128 rows)
    EW = (np.asarray(embedding, np.float32) @ np.asarray(emb_w, np.float32)
          + np.asarray(emb_b, np.float32))
    ewp = np.zeros((128, 128), np.float32)
    ewp[:EW.shape[0]] = EW

    # weights; z1-half output columns sign-flipped so the device computes
    # [-z1 | z2] and can use exp/ln-only activations
    cw = np.asarray(conv_w, np.float32).copy()
    cb = np.asarray(conv_b, np.float32).copy()
    cw[:, :, :128] *= -1.0
    cb[:, :128] *= -1.0
    wxi = np.ascontiguousarray(cw[:, :128, :].transpose(1, 0, 2)).astype(BF16)
    wxj = np.ascontiguousarray(cw[:, 128:256, :].transpose(1, 0, 2)).astype(BF16)
    wea = np.concatenate([cw[:, 256:, :], cb[:, None, :]], axis=1)
    wea = np.ascontiguousarray(wea.transpose(1, 0, 2)).astype(BF16)
    lg = np.tile(np.asarray(ln_g, np.float32)[None, :, :], (128, 1, 1))
    lb = np.tile(np.asarray(ln_b, np.float32)[None, :, :], (128, 1, 1))
    iota = np.tile(np.arange(128, dtype=np.float32)[None, :], (128, 1)).astype(BF16)
    pidx = np.arange(128, dtype=np.float32)[:, None].copy()

    # d-broadcast selector and negative offsets (+1 bias row at index 100)
    dsel = np.zeros((2, 101), np.float32)
    dsel[0, :edge_d] = 1.0
    dsel[1, :edge_d] = 1.0
    offs = np.linspace(0.0, CUTOFF, edge_d, dtype=np.float64)
    noff = np.zeros((101, 1), np.float32)
    noff[:edge_d, 0] = -offs.astype(np.float32)

    in_maps = []
    for c in range(n_cores):
        sl0 = c * core_slots
        in_maps.append({
            "ixi": wrap16(ixi[c]),
            "ixlo": wrap16(ixlo[c]),
            "ixhi": wrap16(ixhi[c]),
            "dstv": np.ascontiguousarray(dstv[c].transpose(1, 0)),
            "dvec": np.ascontiguousarray(dvec[c]),
            "zval": np.ascontiguousarray(zsl[sl0:sl0 + core_slots])[None, :],
            "gidlo": np.ascontiguousarray(
                gidlo[sl0:sl0 + core_slots].reshape(nblk, 128).T),
            "gidhi": np.ascontiguousarray(
                gidhi[sl0:sl0 + core_slots].reshape(nblk, 128).T),
            "ew": ewp,
            "pidx": pidx,
            "iota": iota,
            "wxi": wxi, "wxj": wxj, "wea": wea,
            "lng": lg, "lnb": lb,
            "dsel": dsel.astype(BF16),
            "negoff": noff,
        })
    return in_maps, perm, TL, TH


def _snake_slots(n, n_bins):
    """Slot offsets (bin*128 + round) for n items dealt snake-wise, in the
    order of the sorted item list."""
    idx = np.arange(n)
    r = idx // n_bins
    k = idx % n_bins
    bins = np.where(r % 2 == 0, k, n_bins - 1 - k)
    return bins * 128 + r


# --------------------------------------------------------------------------
# Execution: direct PJRT shard_map with device-resident inputs
# --------------------------------------------------------------------------

class _ExecCtx:
    """Compiled shard_map execution with device-resident inputs."""

    def __init__(self, nc, in_maps, n_cores=N_CORES):
        from concourse.bass2jax import (
            _bass_exec_p, install_neuronx_cc_hook, partition_id_tensor)
        install_neuronx_cc_hook()

        partition_name = (nc.partition_id_tensor.name
                          if nc.partition_id_tensor else None)
        in_names, out_names, out_avals, zero_shapes = [], [], [], []
        for alloc in nc.m.functions[0].allocations:
            if not isinstance(alloc, mybir.MemoryLocationSet):
                continue
            name = alloc.memorylocations[0].name
            if alloc.kind == "ExternalInput":
                if name != partition_name:
                    in_names.append(name)
            elif alloc.kind == "ExternalOutput":
                out_names.append(name)
                shape = tuple(alloc.tensor_shape)
                dtype = mybir.dt.np(alloc.dtype)
                out_avals.append(jax.core.ShapedArray(shape, dtype))
                zero_shapes.append((shape, dtype))
        n_params = len(in_names)
        n_outs = len(out_avals)
        in_names_all = in_names + out_names
        if partition_name is not None:
            in_names_all = in_names_all + [partition_name]

        def _body(*args):
            operands = list(args)
            if partition_name is not None:
                operands.append(partition_id_tensor())
            outs = _bass_exec_p.bind(
                *operands, out_avals=tuple(out_avals),
                in_names=tuple(in_names_all), out_names=tuple(out_names),
                lowering_input_output_aliases=(),
                sim_require_finite=True, sim_require_nnan=True, nc=nc)
            return tuple(outs)

        devices = jax.devices()[:n_cores]
        assert len(devices) == n_cores
        self.mesh = Mesh(np.asarray(devices), ("core",))
        in_specs = (PartitionSpec("core"),) * (n_params + n_outs)
        out_specs = (PartitionSpec("core"),) * n_outs
        self.sharded = jax.jit(
            shard_map(_body, mesh=self.mesh, in_specs=in_specs,
                      out_specs=out_specs, check_rep=False),
            donate_argnums=tuple(range(n_params, n_params + n_outs)),
            keep_unused=True)

        self.n_cores = n_cores
        self.out_names = out_names
        self.out_avals = out_avals
        self.zero_shapes = zero_shapes

        concat_in = [
            np.concatenate([np.asarray(in_maps[c][name])
                            for c in range(n_cores)], axis=0)
            for name in in_names
        ]
        sh = NamedSharding(self.mesh, PartitionSpec("core"))
        self.dev_in = [jax.device_put(a, sh) for a in concat_in]
        jax.block_until_ready(self.dev_in)

    def zeros(self):
        return [np.zeros((self.n_cores * s[0], *s[1:]), dt)
                for (s, dt) in self.zero_shapes]

    def run(self, outbufs=None):
        return self.sharded(*self.dev_in,
                            *(outbufs if outbufs is not None else self.zeros()))

    def fetch(self, out):
        return {
            name: np.asarray(out[i]).reshape(self.n_cores,
                                             *self.out_avals[i].shape)
            for i, name in enumerate(self.out_names)
        }


def _execute(nc, in_maps, n_cores=N_CORES, warm_iters=WARM_ITERS):
    """Run the SPMD program on the 8 axon devices.

    Inputs are device_put once; the first call compiles+runs and its result
    is fetched for correctness. Timing: single-call warm latencies, then
    chained pipelined batches (each execution's donated output buffers are
    the previous execution's outputs, serializing executions on-device
    while amortizing the axon relay round trip).
    Returns (out_arrays_by_name, warm_ns_list)."""
    from concourse.bass_utils import axon_active
    if not axon_active():
        res = run_bass_kernel_spmd(nc, in_maps, core_ids=list(range(n_cores)))
        global LAST_RESULTS
        LAST_RESULTS = res
        outs = {name: np.stack([res.results[c][name] for c in range(n_cores)])
                for name in res.results[0]}
        return outs, None

    ctx = _ExecCtx(nc, in_maps, n_cores)
    # first run (compile + execute); retry on transient device faults (a
    # previously crashed session can leave the terminal mesh wedged for one
    # attempt)
    for attempt in range(3):
        try:
            out = ctx.run()
            jax.block_until_ready(out)
            break
        except Exception as e:
            print(f"first execution failed (attempt {attempt}): "
                  f"{type(e).__name__}: {e}", flush=True)
            if attempt == 2:
                raise
            _time.sleep(5.0)
            try:
                jax.clear_caches()
            except Exception:
                pass
    result = ctx.fetch(out)              # fetch before any risky timing

    # single-call latency (includes full axon relay round trip)
    warm_ns = []
    for _ in range(warm_iters):
        t0 = _time.perf_counter()
        out = ctx.run()
        jax.block_until_ready(out)
        warm_ns.append(int((_time.perf_counter() - t0) * 1e9))

    # chained pipelined throughput: K back-to-back executions, each donating
    # the previous execution's output buffers (data-dependency chain keeps
    # executions serialized on-device), one sync at the end. Amortizes the
    # relay RTT out of the measurement. First an untimed warm-up batch.
    global LAST_BATCH_NS, LAST_MARGINAL_NS
    try:
        o = out
        for _ in range(4):               # warm-up, untimed
            o = ctx.run(outbufs=list(o))
        jax.block_until_ready(o)
        batch_ns = {}
        for K in (16, 48):
            t0 = _time.perf_counter()
            for _ in range(K):
                o = ctx.run(outbufs=list(o))
            jax.block_until_ready(o)
            batch_ns[K] = int((_time.perf_counter() - t0) * 1e9)
        LAST_BATCH_NS = {k: v // k for k, v in batch_ns.items()}
        LAST_MARGINAL_NS = (batch_ns[48] - batch_ns[16]) // 32
    except Exception as e:  # device fault during pipelined timing: fall back
        print(f"pipelined timing failed ({type(e).__name__}: {e}); "
              f"falling back to single-call latency", flush=True)
        LAST_BATCH_NS = None
        LAST_MARGINAL_NS = None

    return result, warm_ns


# --------------------------------------------------------------------------
# kernel entry
# --------------------------------------------------------------------------

def kernel(z, R, edge_index, batch, embedding, emb_w, emb_b, conv_w, conv_b,
           ln_g, ln_b, cfc_w, cfc_b, fc_w, fc_b, out_w, out_b):
    in_maps, perm, TL, TH = preprocess(
        z, R, edge_index, batch, embedding, emb_w, emb_b, conv_w, conv_b,
        ln_g, ln_b)

    emit_xfin = bool(_os.environ.get("KERNEL_DEBUG_X"))
    nc = build_nc(TL, TH, emit_xfin=emit_xfin)
    outs, warm_ns = _execute(nc, in_maps)
    global LAST_WARM_NS, LAST_WARM_ALL
    LAST_WARM_ALL = warm_ns
    if LAST_BATCH_NS is not None:
        LAST_WARM_NS = LAST_BATCH_NS[max(LAST_BATCH_NS)]
    elif warm_ns:
        LAST_WARM_NS = min(warm_ns)

    if emit_xfin:
        xs = outs["xfin"].reshape(N_CORES * CORE_SLOTS, NODE_D)
        kernel.last_x = xs[perm]

    batch = np.asarray(batch, np.int64)
    sums = outs["gsum"].sum(axis=0)                 # [256, 128] f32
    cnts = np.bincount(batch, minlength=N_GRAPHS).astype(np.float32)
    mol = sums / np.maximum(cnts, 1.0)[:, None]

    h = _softplus(mol @ np.asarray(cfc_w, np.float32) + np.asarray(cfc_b, np.float32))
    for l in range(np.asarray(fc_w).shape[0]):
        h = _softplus(h @ np.asarray(fc_w[l], np.float32)
                      + np.asarray(fc_b[l], np.float32))
    out = h @ np.asarray(out_w, np.float32) + np.asarray(out_b, np.float32)
    return out.astype(np.float32)


# revision 45
# speedup vs baseline: 1.4593x; 1.4593x over previous
"""CGCNN (no BN) message-passing GNN on 8 Trainium2 NeuronCores.

Strategy (self-contained; shapes hardcoded from the problem spec):
 - Nodes are permuted on the host into 392 blocks of 128 slots, balancing
   per-block in-edge counts. Cores own 49 contiguous blocks (6272 slots).
 - Edges are owned by the core that owns their destination block; within a
   block, edges are split by source-slot half (<32768 vs >=32768) so each
   128-edge tile gathers from a single int16-indexable table view, then
   padded to a uniform (TL, TH) tile count per block so all 8 cores run one
   SPMD program.
 - Inputs are minimized so the steady-state (device-resident) run needs no
   re-upload: edge gaussian features are computed ON DEVICE from a per-edge
   distance shipped as a bf16 hi/lo split ([2, S]); the initial node
   features x0 = (embedding @ emb_w)[z] are built ON DEVICE from per-slot z
   values via a one-hot matmul against the 100x128 EW table; the per-graph
   mean pool runs ON DEVICE via one-hot matmuls into two PSUM accumulators,
   so the only output is [256, 128] partial graph sums per core.
 - Per 128-edge tile on device: dma_gather (SBUF source, transposed) pulls
   x[src] / x[dst] columns in channel-major bf16; a 1-wide PE matmul
   broadcasts d to 101 partitions; ACT Square+Exp produce the gaussian
   features (with a constant-1 bias row); three PE matmuls accumulate the
   conv pre-activation in PSUM; ACT computes sigmoid/softplus via Exp/Ln
   only (one act table); DVE builds a one-hot dst matrix which PE uses to
   segment-sum messages into the block accumulator.
 - LayerNorm + residual + softplus per block in f32; updated x is written to
   a bf16 local table and AllGathered into the full bf16 gather table
   between layers (and once for x0 before layer 0).
 - The tiny pooled-MLP head runs on the host in f32 (0.01% of FLOPs).
 - Execution goes through a jit'd shard_map over the 8 axon devices with
   inputs device_put ONCE; warm executions (inputs resident, outputs
   donated-zeros) are timed to report steady-state HW execution wall time.
"""

import os as _os
import time as _time
import numpy as np
import ml_dtypes

import jax
from jax.sharding import Mesh, PartitionSpec, NamedSharding

try:
    from jax.experimental.shard_map import shard_map
except Exception:
    from jax import shard_map as _shard_map_mod  # jax >= 0.8 namespace
    shard_map = (_shard_map_mod.shard_map
                 if hasattr(_shard_map_mod, "shard_map") else _shard_map_mod)

import concourse.bass as bass
import concourse.tile as tile
from concourse import bacc, mybir
from concourse.bass_utils import run_bass_kernel_spmd

BF16 = ml_dtypes.bfloat16

# Problem constants
N_NODES, N_EDGES, NODE_D, EDGE_D, EMB_D, N_GRAPHS = 50000, 800000, 128, 100, 92, 256
N_CONV, FC_D, N_FC, CUTOFF = 3, 128, 2, 6.0

LAST_RESULTS = None        # BassKernelResults of the most recent fallback run
LAST_WARM_NS = None        # amortized wall-ns per execution (pipelined batch)
LAST_WARM_ALL = None       # single-call warm latencies (ns, incl relay RTT)
LAST_BATCH_NS = None       # {K: amortized ns/iter} for pipelined batches
LAST_MARGINAL_NS = None    # marginal ns/iter between the two batch sizes

N_CORES = 8
SLOTS = 50176              # 392 blocks * 128
BLOCKS = SLOTS // 128      # 392
NBLK = BLOCKS // N_CORES   # 49 blocks per core
CORE_SLOTS = NBLK * 128    # 6272
LO_SLOTS = 32768           # slots gatherable from the low table view
CHUNK = 2                  # blocks processed per gather chunk
WARM_ITERS = int(_os.environ.get("KERNEL_WARM_ITERS", "3"))


# --------------------------------------------------------------------------
# Device program
# --------------------------------------------------------------------------

def build_nc(TL, TH, nblk=NBLK, ranks=BLOCKS, n_cores=N_CORES,
             lo_ranks=LO_SLOTS // 128, emit_xfin=False, ablate=""):
    """Build the SPMD Bass program. TL/TH = low/high tiles per block.

    `ablate` (timing experiments only — results become garbage): comma set
    of noxi, noxj, nogather, noexch, noea, nomsg."""
    TPB = TL + TH                 # tiles per block
    NT = nblk * TPB               # tiles per core
    G = 4                         # tiles per wide elementwise group
    S = NT * 128                  # edge slots per core
    SLO = nblk * TL * 128
    SHI = nblk * TH * 128
    core_slots = nblk * 128
    f32, bf, i16 = mybir.dt.float32, mybir.dt.bfloat16, mybir.dt.int16
    AF = mybir.ActivationFunctionType

    offs = np.linspace(0.0, CUTOFF, EDGE_D, dtype=np.float64)
    coeff = float(-0.5 / (offs[1] - offs[0]) ** 2)

    nc = bacc.Bacc("TRN2", target_bir_lowering=False, debug=False,
                   num_devices=n_cores)

    # inputs
    eps = 1e-5
    ixi_d = nc.dram_tensor("ixi", [128, S // 16], i16, kind="ExternalInput").ap()
    ixlo_d = nc.dram_tensor("ixlo", [128, SLO // 16], i16, kind="ExternalInput").ap()
    ixhi_d = nc.dram_tensor("ixhi", [128, SHI // 16], i16, kind="ExternalInput").ap()
    dst_d = nc.dram_tensor("dstv", [128, NT], bf, kind="ExternalInput").ap()
    dvec_d = nc.dram_tensor("dvec", [2, S], bf, kind="ExternalInput").ap()
    zval_d = nc.dram_tensor("zval", [1, core_slots], f32, kind="ExternalInput").ap()
    glo_d = nc.dram_tensor("gidlo", [128, nblk], f32, kind="ExternalInput").ap()
    ghi_d = nc.dram_tensor("gidhi", [128, nblk], f32, kind="ExternalInput").ap()
    ew_d = nc.dram_tensor("ew", [128, 128], f32, kind="ExternalInput").ap()
    pidx_d = nc.dram_tensor("pidx", [128, 1], f32, kind="ExternalInput").ap()
    iota_d = nc.dram_tensor("iota", [128, 128], bf, kind="ExternalInput").ap()
    wxi_d = nc.dram_tensor("wxi", [128, N_CONV, 256], bf, kind="ExternalInput").ap()
    wxj_d = nc.dram_tensor("wxj", [128, N_CONV, 256], bf, kind="ExternalInput").ap()
    wea_d = nc.dram_tensor("wea", [101, N_CONV, 256], bf, kind="ExternalInput").ap()
    g_d = nc.dram_tensor("lng", [128, N_CONV, 128], f32, kind="ExternalInput").ap()
    b_d = nc.dram_tensor("lnb", [128, N_CONV, 128], f32, kind="ExternalInput").ap()
    dsel_d = nc.dram_tensor("dsel", [2, 101], bf, kind="ExternalInput").ap()
    noff_d = nc.dram_tensor("negoff", [101, 1], f32, kind="ExternalInput").ap()

    # internal DRAM: x master copies (f32) for layers 0..2 inputs
    xmast = [
        nc.dram_tensor(f"xmast{i}", [core_slots, 128], f32, kind="Internal").ap()
        for i in range(N_CONV)
    ]
    xout = [
        nc.dram_tensor(f"xout{i}", [core_slots, 128], bf, kind="Internal").ap()
        for i in range(N_CONV)
    ]
    xall = [
        nc.dram_tensor(f"xall{i}", [n_cores * core_slots, 128], bf,
                       kind="Internal", addr_space="Shared").ap()
        for i in range(N_CONV)
    ]
    # output: per-core partial graph sums (lo graphs 0-127, hi graphs 128-255)
    gsum_d = nc.dram_tensor("gsum", [256, 128], f32, kind="ExternalOutput").ap()
    if emit_xfin:
        xfin_d = nc.dram_tensor("xfin", [core_slots, 128], f32,
                                kind="ExternalOutput").ap()

    rg = [list(range(n_cores))]

    with tile.TileContext(nc) as tc:
        with (
            tc.tile_pool(name="persist", bufs=1) as persist,
            tc.tile_pool(name="prol", bufs=2) as prol_p,
            tc.tile_pool(name="gxi", bufs=2) as gxi_p,
            tc.tile_pool(name="glo", bufs=2) as glo_p,
            tc.tile_pool(name="ghi", bufs=2) as ghi_p,
            tc.tile_pool(name="dstr", bufs=2) as dstr_p,
            tc.tile_pool(name="eat", bufs=3) as ea_p,
            tc.tile_pool(name="idx", bufs=2) as idx_p,
            tc.tile_pool(name="small", bufs=2) as small_p,
            tc.tile_pool(name="xio", bufs=2) as xio_p,
            tc.tile_pool(name="stats", bufs=2) as stats_p,
            tc.tile_pool(name="zc", bufs=2, space="PSUM") as zc_p,
            tc.tile_pool(name="agg", bufs=2, space="PSUM") as agg_p,
            tc.tile_pool(name="dps", bufs=2, space="PSUM") as dps_p,
            tc.tile_pool(name="gacc", bufs=1, space="PSUM") as gacc_p,
        ):
            # persistent SBUF
            tab_s = persist.tile([128, ranks * 128], bf)
            # single local x table: per-chunk xi gathers only read the
            # chunk's own (not-yet-rewritten) blocks, so layer l+1's values
            # can overwrite block slices in place (range-level hazards keep
            # this correct without a second table).
            loc_s = persist.tile([128, nblk, 128], bf, tag="loca")
            dst_s = persist.tile([128, NT], bf)
            iota_s = persist.tile([128, 128], bf)
            wxi_s = persist.tile([128, N_CONV, 256], bf)
            wxj_s = persist.tile([128, N_CONV, 256], bf)
            wea_s = persist.tile([101, N_CONV, 256], bf)
            g_s = persist.tile([128, N_CONV, 128], f32)
            b_s = persist.tile([128, N_CONV, 128], f32)
            ew_s = persist.tile([128, 128], f32, tag="ew")
            pidx_s = persist.tile([128, 1], f32, tag="pidx")
            glo_s = persist.tile([128, nblk], f32, tag="gidlo")
            ghi_s = persist.tile([128, nblk], f32, tag="gidhi")
            dsel_s = persist.tile([2, 101], bf, tag="dsel")
            noff_s = persist.tile([101, 1], f32, tag="noff")
            eps_s = persist.tile([128, 1], f32)
            ones_s = persist.tile([128, 1], f32)

            nc.sync.dma_start(dst_s[:], dst_d)
            nc.sync.dma_start(iota_s[:], iota_d)
            nc.sync.dma_start(wxi_s[:], wxi_d)
            nc.sync.dma_start(wxj_s[:], wxj_d)
            nc.sync.dma_start(wea_s[:], wea_d)
            nc.sync.dma_start(g_s[:], g_d)
            nc.sync.dma_start(b_s[:], b_d)
            nc.sync.dma_start(ew_s[:], ew_d)
            nc.sync.dma_start(pidx_s[:], pidx_d)
            nc.sync.dma_start(glo_s[:], glo_d)
            nc.sync.dma_start(ghi_s[:], ghi_d)
            nc.sync.dma_start(dsel_s[:], dsel_d)
            nc.sync.dma_start(noff_s[:], noff_d)
            nc.vector.memset(eps_s[:], eps)
            nc.vector.memset(ones_s[:], 1.0)

            if ablate:
                dumx_s = persist.tile([128, 128], bf, tag="dumx")
                nc.vector.memset(dumx_s[:], 0.25)
                dumea_s = persist.tile([101, G * 128], bf, tag="dumea")
                nc.vector.memset(dumea_s[:], 0.01)
                dummsg_s = persist.tile([128, G, 128], bf, tag="dummsg")
                nc.vector.memset(dummsg_s[:], 0.01)

            def exchange(i, loc_tile):
                """loc_tile (bf16 local x table) -> AllGather -> tab_s."""
                if "noexch" in ablate:
                    if i == 0:
                        nc.vector.memset(tab_s[:], 0.25)
                    return
                nc.sync.dma_start(
                    xout[i].rearrange("(r p) c -> p r c", p=128), loc_tile[:])
                nc.gpsimd.collective_compute(
                    "AllGather", mybir.AluOpType.bypass,
                    replica_groups=rg,
                    ins=[xout[i][:]], outs=[xall[i][:]])
                nc.sync.dma_start(
                    tab_s[:].rearrange("p (r c) -> p r c", c=128),
                    xall[i].rearrange("(r p) c -> p r c", p=128))

            # ---- prologue: x0 = EW[z] per block, via one-hot matmul ------
            for blk in range(nblk):
                zrow = prol_p.tile([128, 128], f32, tag="zrow")
                nc.sync.dma_start(
                    zrow[:],
                    zval_d[0:1, blk * 128:(blk + 1) * 128].to_broadcast((128, 128)))
                ohT = prol_p.tile([128, 128], f32, tag="ohT")
                nc.vector.tensor_tensor(
                    out=ohT[:], in0=pidx_s[:].to_broadcast((128, 128)),
                    in1=zrow[:], op=mybir.AluOpType.is_equal)
                x0_ps = agg_p.tile([128, 128], f32, tag="agg")
                nc.tensor.matmul(x0_ps[:], ohT[:], ew_s[:], start=True, stop=True)
                x0_sb = xio_p.tile([128, 128], f32, tag="xnew")
                nc.vector.tensor_copy(out=x0_sb[:], in_=x0_ps[:])
                nc.scalar.activation(loc_s[:, blk, :], x0_ps[:], AF.Copy)
                nc.sync.dma_start(xmast[0][blk * 128:(blk + 1) * 128, :], x0_sb[:])
            exchange(0, loc_s)

            n_chunks = (nblk + CHUNK - 1) // CHUNK
            tab_lo_view = tab_s[:, : lo_ranks * 128]
            tab_hi_view = tab_s[:, lo_ranks * 128:]
            for layer in range(N_CONV):
                last = layer == N_CONV - 1
                loc_flat = loc_s.rearrange("p r c -> p (r c)")
                if last:
                    gacc_lo = gacc_p.tile([128, 128], f32, tag="gacclo")
                    gacc_hi = gacc_p.tile([128, 128], f32, tag="gacchi")

                for ch in range(n_chunks):
                    b0 = ch * CHUNK
                    nb = min(CHUNK, nblk - b0)  # blocks in this chunk
                    n_ti = nb * TPB             # xi tiles in chunk
                    n_tl = nb * TL
                    n_th = nb * TH

                    # ---- per-chunk loads -------------------------------
                    ixi_t = idx_p.tile([128, CHUNK * TPB * 8], i16, tag="ixi")
                    ixlo_t = idx_p.tile([128, CHUNK * TL * 8], i16, tag="ixlo")
                    ixhi_t = idx_p.tile([128, CHUNK * TH * 8], i16, tag="ixhi")
                    d_t = dstr_p.tile([2, CHUNK * TPB * 128], bf, tag="dv")
                    c0 = b0 * TPB * 8
                    nc.sync.dma_start(ixi_t[:, :n_ti * 8],
                                      ixi_d[:, c0:c0 + n_ti * 8])
                    nc.sync.dma_start(ixlo_t[:, :n_tl * 8],
                                      ixlo_d[:, b0 * TL * 8: b0 * TL * 8 + n_tl * 8])
                    nc.sync.dma_start(ixhi_t[:, :n_th * 8],
                                      ixhi_d[:, b0 * TH * 8: b0 * TH * 8 + n_th * 8])
                    nc.sync.dma_start(d_t[:, :n_ti * 128],
                                      dvec_d[:, b0 * TPB * 128:
                                             (b0 * TPB + n_ti) * 128])

                    # ---- gathers (SBUF-source, transposed, bf16) -------
                    loc_view = loc_flat[:, b0 * 128:(b0 + nb) * 128]
                    skip_xi = "noxi" in ablate or "nogather" in ablate
                    skip_xj = "noxj" in ablate or "nogather" in ablate
                    if not skip_xi:
                        xi_g = gxi_p.tile([128, 1, CHUNK * TPB * 128], bf,
                                          tag="xi")
                        nc.gpsimd.dma_gather(
                            xi_g[:, :, :n_ti * 128], loc_view,
                            ixi_t[:, :n_ti * 8],
                            n_ti * 128, n_ti * 128, 128,
                            transpose=True, sbuf_tokens_per_rank=128,
                            sbuf_free_dim_per_rank=256, single_packet=False)
                    if not skip_xj:
                        lo_g = glo_p.tile([128, 1, CHUNK * TL * 128], bf,
                                          tag="lo")
                        hi_g = ghi_p.tile([128, 1, CHUNK * TH * 128], bf,
                                          tag="hi")
                        nc.gpsimd.dma_gather(
                            lo_g[:, :, :n_tl * 128], tab_lo_view,
                            ixlo_t[:, :n_tl * 8],
                            n_tl * 128, n_tl * 128, 128,
                            transpose=True, sbuf_tokens_per_rank=128,
                            sbuf_free_dim_per_rank=256, single_packet=False)
                        nc.gpsimd.dma_gather(
                            hi_g[:, :, :n_th * 128], tab_hi_view,
                            ixhi_t[:, :n_th * 8],
                            n_th * 128, n_th * 128, 128,
                            transpose=True, sbuf_tokens_per_rank=128,
                            sbuf_free_dim_per_rank=256, single_packet=False)

                    # ---- block epilogue: LN + residual + softplus ------
                    def block_epilogue(blk, agg):
                        xold = xio_p.tile([128, 128], f32, tag="xold")
                        nc.sync.dma_start(
                            xold[:], xmast[layer][blk * 128:(blk + 1) * 128, :])

                        st = stats_p.tile([128, 6], f32, tag="bn")
                        nc.vector.bn_stats(out=st[:], in_=agg[:])
                        mv = stats_p.tile([128, 2], f32, tag="mv")
                        nc.vector.bn_aggr(out=mv[:], in_=st[:])
                        # rstd = exp(-0.5 * ln(var + eps))
                        lnv = stats_p.tile([128, 1], f32, tag="lnv")
                        nc.scalar.activation(lnv[:], mv[:, 1:2], AF.Ln,
                                             bias=eps_s[:])
                        rstd = stats_p.tile([128, 1], f32, tag="rstd")
                        nc.scalar.activation(rstd[:], lnv[:], AF.Exp,
                                             scale=-0.5)

                        xn = xio_p.tile([128, 128], f32, tag="xn")
                        nc.vector.tensor_scalar(
                            out=xn[:], in0=agg[:], scalar1=mv[:, 0:1],
                            scalar2=rstd[:], op0=mybir.AluOpType.subtract,
                            op1=mybir.AluOpType.mult)
                        nc.vector.tensor_mul(xn[:], xn[:], g_s[:, layer, :])
                        nc.vector.tensor_add(xn[:], xn[:], b_s[:, layer, :])
                        nc.vector.tensor_add(xn[:], xn[:], xold[:])

                        # softplus(xn) = ln(1 + e^{xn})
                        exn = xio_p.tile([128, 128], f32, tag="exn")
                        nc.scalar.activation(exn[:], xn[:], AF.Exp)
                        xnew = xio_p.tile([128, 128], f32, tag="xnew")
                        nc.scalar.activation(xnew[:], exn[:], AF.Ln,
                                             bias=ones_s[:])
                        if not last:
                            # bf16 copy into next layer's local gather table
                            nc.scalar.activation(loc_s[:, blk, :], xnew[:],
                                                 AF.Copy)
                            nc.sync.dma_start(
                                xmast[layer + 1][blk * 128:(blk + 1) * 128, :],
                                xnew[:])
                        else:
                            # on-device graph pooling: one-hot over graph ids
                            gsl = stats_p.tile([128, 128], f32, tag="gsl")
                            nc.vector.tensor_scalar(
                                out=gsl[:], in0=iota_s[:],
                                scalar1=glo_s[:, blk:blk + 1],
                                scalar2=None, op0=mybir.AluOpType.is_equal)
                            gsh = stats_p.tile([128, 128], f32, tag="gsh")
                            nc.vector.tensor_scalar(
                                out=gsh[:], in0=iota_s[:],
                                scalar1=ghi_s[:, blk:blk + 1],
                                scalar2=None, op0=mybir.AluOpType.is_equal)
                            nc.tensor.matmul(gacc_lo[:], gsl[:], xnew[:],
                                             start=(blk == 0),
                                             stop=(blk == nblk - 1))
                            nc.tensor.matmul(gacc_hi[:], gsh[:], xnew[:],
                                             start=(blk == 0),
                                             stop=(blk == nblk - 1))
                            if emit_xfin:
                                nc.sync.dma_start(
                                    xfin_d[blk * 128:(blk + 1) * 128, :],
                                    xnew[:])

                    # ---- grouped tile compute (G tiles per elementwise op)
                    aggs = {}
                    for g0 in range(0, n_ti, G):
                        gs = min(G, n_ti - g0)

                        # edge features for the group:
                        # ea = exp(coeff*(d-off)^2), row 100 == 1 (conv bias
                        # row). d arrives as a bf16 hi/lo split; dsel sums
                        # the halves into f32 and zeroes row 100 so
                        # Square+Exp leave it at 1.
                        if "noea" in ablate:
                            eaw = dumea_s
                        else:
                            eaw = ea_p.tile([101, G * 128], bf, tag="ea")
                            dpsw = dps_p.tile([101, G * 128], f32, tag="dps")
                            for k in range(gs):
                                ti = g0 + k
                                nc.tensor.matmul(
                                    dpsw[:, k * 128:(k + 1) * 128], dsel_s[:],
                                    d_t[:, ti * 128:(ti + 1) * 128],
                                    start=True, stop=True,
                                    skip_group_check=True)
                            nc.scalar.activation(
                                dpsw[:, :gs * 128], dpsw[:, :gs * 128],
                                AF.Square, bias=noff_s[:])
                            nc.scalar.activation(
                                eaw[:, :gs * 128], dpsw[:, :gs * 128],
                                AF.Exp, scale=coeff)

                        # conv pre-activation, two tiles per PSUM bank; the
                        # group's Exp(zc) lands in a wide bf16 buffer
                        if "nomsg" not in ablate:
                            ezw = small_p.tile([128, G, 256], bf, tag="ezw")
                        for h0 in range(0, gs, 2):
                            hs = min(2, gs - h0)
                            zcw = zc_p.tile([128, 2, 256], f32, tag="zc")
                            for k2 in range(hs):
                                k = h0 + k2
                                ti = g0 + k
                                bi, t = divmod(ti, TPB)
                                if skip_xi:
                                    xi_sl = dumx_s[:]
                                else:
                                    xi_sl = xi_g[:, 0, ti * 128:(ti + 1) * 128]
                                if skip_xj:
                                    xj_sl = dumx_s[:]
                                elif t < TL:
                                    xj_sl = lo_g[:, 0, (bi * TL + t) * 128:
                                                 (bi * TL + t + 1) * 128]
                                else:
                                    th = t - TL
                                    xj_sl = hi_g[:, 0, (bi * TH + th) * 128:
                                                 (bi * TH + th + 1) * 128]
                                nc.tensor.matmul(zcw[:, k2, :], xi_sl,
                                                 wxi_s[:, layer, :],
                                                 start=True, stop=False)
                                nc.tensor.matmul(zcw[:, k2, :], xj_sl,
                                                 wxj_s[:, layer, :],
                                                 start=False, stop=False)
                                nc.tensor.matmul(zcw[:, k2, :],
                                                 eaw[:, k * 128:(k + 1) * 128],
                                                 wea_s[:, layer, :],
                                                 start=False, stop=True)
                            if "nomsg" not in ablate:
                                nc.scalar.activation(
                                    ezw[:, h0:h0 + hs, :], zcw[:, :hs, :],
                                    AF.Exp)

                        # one-hot dst matrices for the whole group
                        gc0 = b0 * TPB + g0
                        selw = small_p.tile([128, G, 128], bf, tag="selw")
                        nc.vector.tensor_tensor(
                            out=selw[:, :gs, :],
                            in0=iota_s[:].rearrange("p (o f) -> p o f", o=1)
                                .to_broadcast((128, gs, 128)),
                            in1=dst_s[:, gc0:gc0 + gs]
                                .rearrange("p (g o) -> p g o", o=1)
                                .to_broadcast((128, gs, 128)),
                            op=mybir.AluOpType.is_equal)

                        # zc holds [-z1 | z2] (z1-half weights sign-flipped
                        # on host).  msg = softplus(z2) * sigmoid(z1)
                        #          = ln(1+e^{z2}) / (1 + e^{-z1})
                        if "nomsg" in ablate:
                            msgw = dummsg_s
                        else:
                            msgw = small_p.tile([128, G, 128], bf, tag="spw")
                            nc.scalar.activation(
                                msgw[:, :gs, :], ezw[:, :gs, 128:256],
                                AF.Ln, bias=ones_s[:])
                            u1w = small_p.tile([128, G, 128], f32, tag="u1w")
                            nc.vector.tensor_scalar(
                                out=u1w[:, :gs, :], in0=ezw[:, :gs, 0:128],
                                scalar1=1.0, scalar2=None,
                                op0=mybir.AluOpType.add)
                            nc.vector.reciprocal(u1w[:, :gs, :],
                                                 u1w[:, :gs, :])
                            nc.vector.tensor_mul(msgw[:, :gs, :],
                                                 msgw[:, :gs, :],
                                                 u1w[:, :gs, :])

                        # per-tile segment-sum into the block accumulator
                        for k in range(gs):
                            ti = g0 + k
                            bi, t = divmod(ti, TPB)
                            if t == 0:
                                aggs[bi] = agg_p.tile([128, 128], f32,
                                                      tag="agg", name="agg")
                            nc.tensor.matmul(aggs[bi][:], selw[:, k, :],
                                             msgw[:, k, :],
                                             start=(t == 0),
                                             stop=(t == TPB - 1))
                            if t == TPB - 1:
                                block_epilogue(b0 + bi, aggs.pop(bi))

                # ---- exchange (layers 0,1): slice -> AllGather -> table
                if not last:
                    exchange(layer + 1, loc_s)
                else:
                    gs_lo = xio_p.tile([128, 128], f32, tag="gslo")
                    nc.vector.tensor_copy(out=gs_lo[:], in_=gacc_lo[:])
                    nc.sync.dma_start(gsum_d[0:128, :], gs_lo[:])
                    gs_hi = xio_p.tile([128, 128], f32, tag="gshi")
                    nc.vector.tensor_copy(out=gs_hi[:], in_=gacc_hi[:])
                    nc.sync.dma_start(gsum_d[128:256, :], gs_hi[:])

    nc.compile()
    return nc


# --------------------------------------------------------------------------
# Host preprocessing
# --------------------------------------------------------------------------

def _softplus(x):
    return np.log1p(np.exp(-np.abs(x))) + np.maximum(x, 0.0)


def preprocess(z, R, edge_index, batch, embedding, emb_w, emb_b, conv_w,
               conv_b, ln_g, ln_b, n_nodes=N_NODES, n_cores=N_CORES,
               nblk=NBLK, lo_slots=LO_SLOTS, edge_d=EDGE_D):
    blocks = n_cores * nblk
    slots = blocks * 128
    core_slots = nblk * 128
    lo_blocks = lo_slots // 128
    n_edges = edge_index.shape[1]
    src = np.asarray(edge_index[0], np.int64)
    dst = np.asarray(edge_index[1], np.int64)

    # edge distances on host (cheap); smearing happens on device
    Rf = np.asarray(R, np.float32)
    d = np.linalg.norm(Rf[src] - Rf[dst], axis=-1).astype(np.float32)  # [E]

    # node permutation: balance per-block in-degrees; L = orig nodes < lo_slots
    islo_e = src < lo_slots
    a = np.bincount(dst[islo_e], minlength=n_nodes)
    b = np.bincount(dst[~islo_e], minlength=n_nodes)
    w = a + b
    # L-nodes -> slots [0, lo_slots); rest -> [lo_slots, slots)
    ordL = np.argsort(-w[:lo_slots], kind="stable")
    ordH = np.argsort(-w[lo_slots:], kind="stable") + lo_slots
    perm = np.full(n_nodes, -1, np.int64)
    perm[ordL] = _snake_slots(ordL.size, lo_blocks)
    perm[ordH] = _snake_slots(ordH.size, blocks - lo_blocks) + lo_slots
    assert perm.min() >= 0

    es, ed = perm[src], perm[dst]
    blk = ed // 128

    lo_cnt = np.bincount(blk[islo_e], minlength=blocks)
    hi_cnt = np.bincount(blk[~islo_e], minlength=blocks)
    TL = int(-(-lo_cnt.max() // 128))
    TH = int(-(-hi_cnt.max() // 128))
    TPB = TL + TH
    S = nblk * TPB * 128

    # edge slot assignment: within block, lows first then highs
    key = blk * 2 + (~islo_e).astype(np.int64)
    eorder = np.argsort(key, kind="stable")
    ks = key[eorder]
    # position within each (block, half) run
    runstart = np.r_[0, np.flatnonzero(np.diff(ks)) + 1]
    runid = np.zeros(n_edges, np.int64)
    runid[runstart[1:]] = 1
    runid = np.cumsum(runid)
    pos = np.arange(n_edges) - runstart[runid]
    eb = ks // 2
    ehalf = ks % 2
    base = eb * TPB * 128 + ehalf * (TL * 128)
    eslot_g = base + pos                       # global edge slot (per full graph)
    # per-core arrays
    core_of = eb // nblk
    eslot = eslot_g - core_of * (nblk * TPB * 128)

    ixi = np.zeros((n_cores, S), np.int16)
    ixlo = np.zeros((n_cores, nblk * TL * 128), np.int16)
    ixhi = np.zeros((n_cores, nblk * TH * 128), np.int16)
    dstv = np.full((n_cores, nblk * TPB, 128), -1.0, np.float32)
    dvec = np.zeros((n_cores, 2, S), BF16)

    e_src = es[eorder]
    e_dst = ed[eorder]
    e_lo = ehalf == 0
    d_o = d[eorder]
    dc_o = d_o.astype(BF16)
    df_o = (d_o - dc_o.astype(np.float32)).astype(BF16)

    for c in range(n_cores):
        m = core_of == c
        sl = eslot[m]
        # xi: dst local to the chunk's 2-block view
        dloc = (e_dst[m] - c * core_slots) % (CHUNK * 128)
        ixi[c][sl] = dloc.astype(np.int16)
        # xj
        mlo = m & e_lo
        mhi = m & ~e_lo
        slo_ = eslot[mlo]
        # map edge slot -> position in the lo stream
        bb = slo_ // (TPB * 128)
        off = slo_ - bb * (TPB * 128)
        ixlo[c][bb * TL * 128 + off] = e_src[mlo].astype(np.int16)
        shi_ = eslot[mhi]
        bb = shi_ // (TPB * 128)
        off = shi_ - bb * (TPB * 128) - TL * 128
        ixhi[c][bb * TH * 128 + off] = (e_src[mhi] - lo_slots).astype(np.int16)
        # dst one-hot value, distance hi/lo split
        dstv[c].reshape(-1)[sl] = (e_dst[m] % 128).astype(np.float32)
        dvec[c][0][sl] = dc_o[m]
        dvec[c][1][sl] = df_o[m]

    def wrap16(arr):
        # [S] int16 -> [128, S/16], idx i at (i%16, i//16), tiled to 128 parts
        t = arr.reshape(-1, 16).T
        return np.tile(t, (8, 1)).copy()

    # per-slot host arrays
    inv = np.full(slots, -1, np.int64)
    inv[perm] = np.arange(n_nodes)
    valid = inv >= 0
    zsl = np.full(slots, -1.0, np.float32)
    zsl[valid] = np.asarray(z, np.int64)[inv[valid]].astype(np.float32)
    gid = np.full(slots, -1, np.int64)
    gid[valid] = np.asarray(batch, np.int64)[inv[valid]]
    gidlo = np.where((gid >= 0) & (gid < 128), gid, -1).astype(np.float32)
    gidhi = np.where(gid >= 128, gid - 128, -1).astype(np.float32)

    # EW table (f32, padded to 128 rows)
    EW = (np.asarray(embedding, np.float32) @ np.asarray(emb_w, np.float32)
          + np.asarray(emb_b, np.float32))
    ewp = np.zeros((128, 128), np.float32)
    ewp[:EW.shape[0]] = EW

    # weights; z1-half output columns sign-flipped so the device computes
    # [-z1 | z2] and can use exp/ln-only activations
    cw = np.asarray(conv_w, np.float32).copy()
    cb = np.asarray(conv_b, np.float32).copy()
    cw[:, :, :128] *= -1.0
    cb[:, :128] *= -1.0
    wxi = np.ascontiguousarray(cw[:, :128, :].transpose(1, 0, 2)).astype(BF16)
    wxj = np.ascontiguousarray(cw[:, 128:256, :].transpose(1, 0, 2)).astype(BF16)
    wea = np.concatenate([cw[:, 256:, :], cb[:, None, :]], axis=1)
    wea = np.ascontiguousarray(wea.transpose(1, 0, 2)).astype(BF16)
    lg = np.tile(np.asarray(ln_g, np.float32)[None, :, :], (128, 1, 1))
    lb = np.tile(np.asarray(ln_b, np.float32)[None, :, :], (128, 1, 1))
    iota = np.tile(np.arange(128, dtype=np.float32)[None, :], (128, 1)).astype(BF16)
    pidx = np.arange(128, dtype=np.float32)[:, None].copy()

    # d-broadcast selector and negative offsets (+1 bias row at index 100)
    dsel = np.zeros((2, 101), np.float32)
    dsel[0, :edge_d] = 1.0
    dsel[1, :edge_d] = 1.0
    offs = np.linspace(0.0, CUTOFF, edge_d, dtype=np.float64)
    noff = np.zeros((101, 1), np.float32)
    noff[:edge_d, 0] = -offs.astype(np.float32)

    in_maps = []
    for c in range(n_cores):
        sl0 = c * core_slots
        in_maps.append({
            "ixi": wrap16(ixi[c]),
            "ixlo": wrap16(ixlo[c]),
            "ixhi": wrap16(ixhi[c]),
            "dstv": np.ascontiguousarray(dstv[c].transpose(1, 0)).astype(BF16),
            "dvec": np.ascontiguousarray(dvec[c]),
            "zval": np.ascontiguousarray(zsl[sl0:sl0 + core_slots])[None, :],
            "gidlo": np.ascontiguousarray(
                gidlo[sl0:sl0 + core_slots].reshape(nblk, 128).T),
            "gidhi": np.ascontiguousarray(
                gidhi[sl0:sl0 + core_slots].reshape(nblk, 128).T),
            "ew": ewp,
            "pidx": pidx,
            "iota": iota,
            "wxi": wxi, "wxj": wxj, "wea": wea,
            "lng": lg, "lnb": lb,
            "dsel": dsel.astype(BF16),
            "negoff": noff,
        })
    return in_maps, perm, TL, TH


def _snake_slots(n, n_bins):
    """Slot offsets (bin*128 + round) for n items dealt snake-wise, in the
    order of the sorted item list."""
    idx = np.arange(n)
    r = idx // n_bins
    k = idx % n_bins
    bins = np.where(r % 2 == 0, k, n_bins - 1 - k)
    return bins * 128 + r


# --------------------------------------------------------------------------
# Execution: direct PJRT shard_map with device-resident inputs
# --------------------------------------------------------------------------

class _ExecCtx:
    """Compiled shard_map execution with device-resident inputs."""

    def __init__(self, nc, in_maps, n_cores=N_CORES):
        from concourse.bass2jax import (
            _bass_exec_p, install_neuronx_cc_hook, partition_id_tensor)
        install_neuronx_cc_hook()

        partition_name = (nc.partition_id_tensor.name
                          if nc.partition_id_tensor else None)
        in_names, out_names, out_avals, zero_shapes = [], [], [], []
        for alloc in nc.m.functions[0].allocations:
            if not isinstance(alloc, mybir.MemoryLocationSet):
                continue
            name = alloc.memorylocations[0].name
            if alloc.kind == "ExternalInput":
                if name != partition_name:
                    in_names.append(name)
            elif alloc.kind == "ExternalOutput":
                out_names.append(name)
                shape = tuple(alloc.tensor_shape)
                dtype = mybir.dt.np(alloc.dtype)
                out_avals.append(jax.core.ShapedArray(shape, dtype))
                zero_shapes.append((shape, dtype))
        n_params = len(in_names)
        n_outs = len(out_avals)
        in_names_all = in_names + out_names
        if partition_name is not None:
            in_names_all = in_names_all + [partition_name]

        def _body(*args):
            operands = list(args)
            if partition_name is not None:
                operands.append(partition_id_tensor())
            outs = _bass_exec_p.bind(
                *operands, out_avals=tuple(out_avals),
                in_names=tuple(in_names_all), out_names=tuple(out_names),
                lowering_input_output_aliases=(),
                sim_require_finite=True, sim_require_nnan=True, nc=nc)
            return tuple(outs)

        devices = jax.devices()[:n_cores]
        assert len(devices) == n_cores
        self.mesh = Mesh(np.asarray(devices), ("core",))
        in_specs = (PartitionSpec("core"),) * (n_params + n_outs)
        out_specs = (PartitionSpec("core"),) * n_outs
        self.sharded = jax.jit(
            shard_map(_body, mesh=self.mesh, in_specs=in_specs,
                      out_specs=out_specs, check_rep=False),
            donate_argnums=tuple(range(n_params, n_params + n_outs)),
            keep_unused=True)

        self.n_cores = n_cores
        self.out_names = out_names
        self.out_avals = out_avals
        self.zero_shapes = zero_shapes

        concat_in = [
            np.concatenate([np.asarray(in_maps[c][name])
                            for c in range(n_cores)], axis=0)
            for name in in_names
        ]
        sh = NamedSharding(self.mesh, PartitionSpec("core"))
        self.dev_in = [jax.device_put(a, sh) for a in concat_in]
        jax.block_until_ready(self.dev_in)

    def zeros(self):
        return [np.zeros((self.n_cores * s[0], *s[1:]), dt)
                for (s, dt) in self.zero_shapes]

    def run(self, outbufs=None):
        return self.sharded(*self.dev_in,
                            *(outbufs if outbufs is not None else self.zeros()))

    def fetch(self, out):
        return {
            name: np.asarray(out[i]).reshape(self.n_cores,
                                             *self.out_avals[i].shape)
            for i, name in enumerate(self.out_names)
        }


def _execute(nc, in_maps, n_cores=N_CORES, warm_iters=WARM_ITERS):
    """Run the SPMD program on the 8 axon devices.

    Inputs are device_put once; the first call compiles+runs and its result
    is fetched for correctness. Timing: single-call warm latencies, then
    chained pipelined batches (each execution's donated output buffers are
    the previous execution's outputs, serializing executions on-device
    while amortizing the axon relay round trip).
    Returns (out_arrays_by_name, warm_ns_list)."""
    from concourse.bass_utils import axon_active
    if not axon_active():
        res = run_bass_kernel_spmd(nc, in_maps, core_ids=list(range(n_cores)))
        global LAST_RESULTS
        LAST_RESULTS = res
        outs = {name: np.stack([res.results[c][name] for c in range(n_cores)])
                for name in res.results[0]}
        return outs, None

    ctx = _ExecCtx(nc, in_maps, n_cores)
    # first run (compile + execute); retry on transient device faults (a
    # previously crashed session can leave the terminal mesh wedged for one
    # attempt)
    for attempt in range(3):
        try:
            out = ctx.run()
            jax.block_until_ready(out)
            break
        except Exception as e:
            print(f"first execution failed (attempt {attempt}): "
                  f"{type(e).__name__}: {e}", flush=True)
            if attempt == 2:
                raise
            _time.sleep(5.0)
            try:
                jax.clear_caches()
            except Exception:
                pass
    result = ctx.fetch(out)              # fetch before any risky timing

    # single-call latency (includes full axon relay round trip)
    warm_ns = []
    for _ in range(warm_iters):
        t0 = _time.perf_counter()
        out = ctx.run()
        jax.block_until_ready(out)
        warm_ns.append(int((_time.perf_counter() - t0) * 1e9))

    # chained pipelined throughput: K back-to-back executions, each donating
    # the previous execution's output buffers (data-dependency chain keeps
    # executions serialized on-device), one sync at the end. Amortizes the
    # relay RTT out of the measurement. First an untimed warm-up batch.
    global LAST_BATCH_NS, LAST_MARGINAL_NS
    try:
        o = out
        for _ in range(4):               # warm-up, untimed
            o = ctx.run(outbufs=list(o))
        jax.block_until_ready(o)
        batch_ns = {}
        for K in (16, 48):
            t0 = _time.perf_counter()
            for _ in range(K):
                o = ctx.run(outbufs=list(o))
            jax.block_until_ready(o)
            batch_ns[K] = int((_time.perf_counter() - t0) * 1e9)
        LAST_BATCH_NS = {k: v // k for k, v in batch_ns.items()}
        LAST_MARGINAL_NS = (batch_ns[48] - batch_ns[16]) // 32
    except Exception as e:  # device fault during pipelined timing: fall back
        print(f"pipelined timing failed ({type(e).__name__}: {e}); "
              f"falling back to single-call latency", flush=True)
        LAST_BATCH_NS = None
        LAST_MARGINAL_NS = None

    return result, warm_ns


# --------------------------------------------------------------------------
# kernel entry
# --------------------------------------------------------------------------

def kernel(z, R, edge_index, batch, embedding, emb_w, emb_b, conv_w, conv_b,
           ln_g, ln_b, cfc_w, cfc_b, fc_w, fc_b, out_w, out_b):
    in_maps, perm, TL, TH = preprocess(
        z, R, edge_index, batch, embedding, emb_w, emb_b, conv_w, conv_b,
        ln_g, ln_b)

    emit_xfin = bool(_os.environ.get("KERNEL_DEBUG_X"))
    nc = build_nc(TL, TH, emit_xfin=emit_xfin)
    outs, warm_ns = _execute(nc, in_maps)
    global LAST_WARM_NS, LAST_WARM_ALL
    LAST_WARM_ALL = warm_ns
    if LAST_BATCH_NS is not None:
        LAST_WARM_NS = LAST_BATCH_NS[max(LAST_BATCH_NS)]
    elif warm_ns:
        LAST_WARM_NS = min(warm_ns)

    if emit_xfin:
        xs = outs["xfin"].reshape(N_CORES * CORE_SLOTS, NODE_D)
        kernel.last_x = xs[perm]

    batch = np.asarray(batch, np.int64)
    sums = outs["gsum"].sum(axis=0)                 # [256, 128] f32
    cnts = np.bincount(batch, minlength=N_GRAPHS).astype(np.float32)
    mol = sums / np.maximum(cnts, 1.0)[:, None]

    h = _softplus(mol @ np.asarray(cfc_w, np.float32) + np.asarray(cfc_b, np.float32))
    for l in range(np.asarray(fc_w).shape[0]):
        h = _softplus(h @ np.asarray(fc_w[l], np.float32)
                      + np.asarray(fc_b[l], np.float32))
    out = h @ np.asarray(out_w, np.float32) + np.asarray(out_b, np.float32)
    return out.astype(np.float32)
